# revision 1
# baseline (speedup 1.0000x reference)
"""nn_DWTFrontEnd Trainium2 Bass kernel.

kernel(x) -> 4-band tuple; 3-level db4 DWT per-band reconstruction.
Shards the 2048 signals across 8 NeuronCores (embarrassingly data
parallel), 128 signals per core per launch, 2 sequential launches.
"""
import sys
for p in ("/opt/trn_rl_repo", "/root/.axon_site/_ro/trn_rl_repo"):
    if p not in sys.path:
        sys.path.append(p)

import numpy as np
import concourse.bass as bass
import concourse.mybir as mybir
import concourse.tile as tile
from concourse.tile_rust import add_dep_helper

F32 = mybir.dt.float32
ALU = mybir.AluOpType

REC_LO = np.array([0.23037781330885523, 0.7148465705525415, 0.6308807679295904,
                   -0.027983769416983849, -0.18703481171888114, 0.030841381835986965,
                   0.032883011666982945, -0.010597401784997278], dtype=np.float32)
F = 8
REC_HI = np.array([(-1.0) ** k * REC_LO[F - 1 - k] for k in range(F)], dtype=np.float32)
DEC_LO = REC_LO[::-1].copy()
DEC_HI = REC_HI[::-1].copy()
TAPS = np.concatenate([REC_LO, REC_HI])
N = 8192
L1, L2, L3 = 4099, 2053, 1030
CHUNK = 512


def _diags_np():
    d = np.zeros((128, 16 * 128), dtype=np.float32)
    eye = np.eye(128, dtype=np.float32)
    for j in range(16):
        d[:, j * 128:(j + 1) * 128] = eye * TAPS[j]
    return d


def build_kernel():
    nc = bass.Bass(trn_type="TRN2")
    x_d = nc.dram_tensor("x", [128, N], F32, kind="ExternalInput").ap()
    y_d = nc.dram_tensor("y", [4, 128, N], F32, kind="ExternalOutput").ap()
    dg_d = nc.inline_tensor(_diags_np(), name="diags").ap()

    sinks = []
    last = {"act": None, "dve": None}
    with tile.TileContext(nc) as tc:
        with tc.tile_pool(name="ded", bufs=1) as ded, \
             tc.tile_pool(name="work", bufs=2) as work, \
             tc.tile_pool(name="psum", bufs=6, space="PSUM") as psum:

            # ---------- loads (dedicated, never-recycled slots) ----------
            x_raw = ded.tile([128, N], F32, tag="xin")
            sinks.append(nc.sync.dma_start(x_raw, x_d))

            # ---------- helpers ----------
            def dve_chain(out_sl, srcs, coefs, assist=False):
                # assist: tap0 on ACT. Only legal when the buffer's readers
                # tolerate a 2-proc writer set (never for DMA-stored tiles).
                if assist:
                    last["act"] = nc.scalar.mul(out_sl, srcs[0],
                                                float(coefs[0]))
                else:
                    last["dve"] = nc.vector.tensor_scalar_mul(
                        out_sl, srcs[0], float(coefs[0]))
                for s, cf in zip(srcs[1:], coefs[1:]):
                    last["dve"] = nc.vector.scalar_tensor_tensor(
                        out_sl, s, float(cf), out_sl, ALU.mult, ALU.add)

            def dve_analysis_raw(dst, dst_off, width, src, filt):
                # src has NO symmetric pads: interior outputs only
                # (dst[dst_off+i] = sum_k h[k] src[2i+k-6] valid for
                #  2i+k-6 in range). Boundary columns handled separately.
                h = REC_LO if filt == "lo" else REC_HI
                srcs = [src[:, k - 6 + 2 * 3: k - 6 + 2 * 3 + 2 * (width - 6) - 1: 2]
                        for k in range(8)]
                dve_chain(dst[:, dst_off + 3:dst_off + width - 3], srcs, list(h))

            def dve_analysis(dst, dst_off, width, srcpad, filt, assist=False):
                # srcpad starts at the padded origin (6 left pads present)
                h = REC_LO if filt == "lo" else REC_HI
                srcs = [srcpad[:, k: k + 2 * (width - 1) + 1: 2] for k in range(8)]
                dve_chain(dst[:, dst_off:dst_off + width], srcs, list(h),
                          assist=assist)

            def dve_synth(y, a, g_name, L, assist=False):
                g = DEC_LO if g_name == "lo" else DEC_HI
                W = L - 3
                for r in (0, 1):
                    taps = [(2 * m + 1) if r == 0 else (2 * m) for m in range(4)]
                    srcs = [a[:, m:m + W] for m in range(4)]
                    dve_chain(y[:, r:2 * W:2], srcs, [g[t] for t in taps],
                              assist=assist)

            def dve_pads(buf, off, width):
                nc.vector.tensor_copy(buf[:, 0:off],
                                      buf[:, off + off - 1: off - 1: -1])
                e = off + width
                nc.vector.tensor_copy(buf[:, e:e + 7], buf[:, e - 1: e - 8: -1])

            def pe_analysis(dst, dst_off, srcpad, filt, width):
                base = 0 if filt == "lo" else 8
                for c in range(0, width, CHUNK):
                    w = min(CHUNK, width - c)
                    ps = psum.tile([128, CHUNK], F32, tag="ps")
                    for k in range(8):
                        rhs = srcpad[:, k + 2 * c: k + 2 * c + 2 * (w - 1) + 1: 2]
                        nc.tensor.matmul(ps[:, :w],
                                         dgl[:, (base + k) * 128:(base + k + 1) * 128],
                                         rhs, start=(k == 0), stop=(k == 7))
                    nc.vector.tensor_copy(dst[:, dst_off + c:dst_off + c + w],
                                          ps[:, :w])

            # ---------- A1 on DVE straight from x_raw ----------
            # interior: dst[6+i] for i in [3, L1-4]: reads x[2i+k-6] all valid
            cA1e = ded.tile([128, L1 + 13], F32, tag="cA1e")
            cD1 = ded.tile([128, L1 + 13], F32, tag="cD1")
            for dst, filt in ((cA1e, "lo"), (cD1, "hi")):
                h = REC_LO if filt == "lo" else REC_HI
                W = L1 - 6
                srcs = [x_raw[:, 2 * 3 + k - 6: 2 * 3 + k - 6 + 2 * (W - 1) + 1: 2]
                        for k in range(8)]
                dve_chain(dst[:, 6 + 3:6 + 3 + W], srcs, list(h))
                # boundary outputs i in {0,1,2} and {L1-3..L1-1}: symmetric
                # extension folds to reversed x slices; do per-tap 1-col ops
                for i in (0, 1, 2, L1 - 3, L1 - 2, L1 - 1):
                    col = dst[:, 6 + i:6 + i + 1]
                    first = True
                    for k in range(8):
                        j = 2 * i + k - 6
                        if j < 0:
                            j = -1 - j          # reference: xe[p]=x[5-p], p=6+j... folds to -1-j
                        elif j >= N:
                            j = 2 * N - 1 - j
                        src = x_raw[:, j:j + 1]
                        if first:
                            nc.vector.tensor_scalar_mul(col, src, float(h[k]))
                            first = False
                        else:
                            nc.vector.scalar_tensor_tensor(
                                col, src, float(h[k]), col, ALU.mult, ALU.add)
            dve_pads(cA1e, 6, L1)

            # ---------- A2 (DVE) ----------
            cA2e = ded.tile([128, L2 + 13], F32, tag="cA2e")
            dve_analysis(cA2e, 6, L2, cA1e, "lo", assist=False)
            dve_pads(cA2e, 6, L2)
            cD2 = work.tile([128, L2 + 13], F32, tag="work")
            dve_analysis(cD2, 0, L2, cA1e, "hi", assist=False)

            # ---------- A3 (DVE, cD3 only) ----------
            cD3 = ded.tile([128, L3], F32, tag="cD3")
            dve_analysis(cD3, 0, L3, cA2e, "hi", assist=False)

            # ---------- band3 ----------
            y3 = ded.tile([128, N], F32, tag="y3")
            dve_synth(y3, cD1[:, 6:6 + L1], "hi", L1)
            sinks.append(nc.sync.dma_start(y_d[3], y3))
            nc.vector.tensor_tensor(x_raw, x_raw, y3, ALU.subtract)

            # ---------- band2 ----------
            t2 = work.tile([128, 2 * L2 - 6], F32, tag="work")
            dve_synth(t2, cD2[:, :L2], "hi", L2)
            y2 = ded.tile([128, N], F32, tag="y2")
            dve_synth(y2, t2[:, :L1], "lo", L1)
            sinks.append(nc.sync.dma_start(y_d[2], y2))
            nc.vector.tensor_tensor(x_raw, x_raw, y2, ALU.subtract)

            # ---------- band1 ----------
            t3 = work.tile([128, 2 * L3 - 6], F32, tag="work")
            dve_synth(t3, cD3[:, :L3], "hi", L3, assist=True)
            t2b = work.tile([128, 2 * L2 - 6], F32, tag="work")
            dve_synth(t2b, t3[:, :L2], "lo", L2)
            y1 = ded.tile([128, N], F32, tag="y1")
            dve_synth(y1, t2b[:, :L1], "lo", L1)
            sinks.append(nc.sync.dma_start(y_d[1], y1))
            nc.vector.tensor_tensor(x_raw, x_raw, y1, ALU.subtract)

            # ---------- band0 = accumulated x_raw ----------
            sinks.append(nc.sync.dma_start(y_d[0], x_raw))

            # tail: absorb every outstanding proc onto SP, one nop each
            tc.no_sync_barrier()
            for s in sinks + [v for v in last.values() if v is not None]:
                n = nc.sync.nop()
                add_dep_helper(n.ins, s.ins, reason="tail absorb")
    return nc


_NC_CACHE = None


def run_full(x_full, trace=False):
    """x_full: (32, 64, 8192) f32 -> tuple of 4 bands, each (32,64,8192)."""
    from concourse.bass_utils import run_bass_kernel_spmd
    global _NC_CACHE
    B, C, n = x_full.shape
    xf = np.ascontiguousarray(x_full.reshape(B * C, n).astype(np.float32))
    n_cores = 8
    if _NC_CACHE is None:
        _NC_CACHE = build_kernel()
    nc = _NC_CACHE
    bands = np.empty((4, B * C, n), dtype=np.float32)
    nrun = xf.shape[0] // (n_cores * 128)
    res = None
    for r in range(nrun):
        base = r * n_cores * 128
        in_maps = [{"x": xf[base + i * 128: base + (i + 1) * 128]}
                   for i in range(n_cores)]
        res = run_bass_kernel_spmd(nc, in_maps, core_ids=list(range(n_cores)),
                                   trace=trace)
        for i in range(n_cores):
            bands[:, base + i * 128: base + (i + 1) * 128, :] = \
                res.results[i]["y"]
    out = tuple(bands[j].reshape(B, C, n) for j in range(4))
    return out, res


def kernel(x):
    out, _ = run_full(np.asarray(x))
    return out



# revision 6
# speedup vs baseline: 6.1299x; 6.1299x over previous
"""nn_DWTFrontEnd Trainium2 Bass kernel (v2: PE-matmul, position-major).

3-level db4 DWT band split/reconstruction of 2048 signals x 8192.
Sharding: 256 signals per core (8 cores). Inside each core, signals are
laid out position-major ([128 positions, 256 signals] blocks) so every
conv stage becomes banded-matrix matmuls on the PE engine (fp16 data,
fp32 PSUM accumulate). Synthesis uses per-band composed operators.
DVE/ACT/Pool split the PSUM->SBUF copies; host transposes shards.
"""
import sys
for p in ("/opt/trn_rl_repo", "/root/.axon_site/_ro/trn_rl_repo"):
    if p not in sys.path:
        sys.path.append(p)

import numpy as np
import concourse.bass as bass
import concourse.mybir as mybir
import concourse.tile as tile
from concourse.tile_rust import add_dep_helper

F32 = mybir.dt.float32
F16 = mybir.dt.float16

REC_LO = np.array([0.23037781330885523, 0.7148465705525415, 0.6308807679295904,
                   -0.027983769416983849, -0.18703481171888114, 0.030841381835986965,
                   0.032883011666982945, -0.010597401784997278], dtype=np.float64)
F = 8
REC_HI = np.array([(-1.0) ** k * REC_LO[F - 1 - k] for k in range(F)], dtype=np.float64)
DEC_LO = REC_LO[::-1].copy()
DEC_HI = REC_HI[::-1].copy()

N = 8192
L1, L2, L3 = 4099, 2053, 1030
S = 256            # signals per core
P = 128            # positions per block


def _dwt_apply(X, filt):
    n = X.shape[1]
    idx = np.pad(np.arange(n), (F - 1, F - 1), mode='symmetric')[1:]
    Xe = X[:, idx]
    L = (n + 13 - F) // 2 + 1
    out = np.zeros((X.shape[0], L), dtype=X.dtype)
    for k in range(F):
        out += filt[k] * Xe[:, k:k + 2 * (L - 1) + 1:2]
    return out


def _idwt_half_apply(A, filt):
    B, L = A.shape
    n_out = 2 * L - 6
    out = np.zeros((B, n_out), dtype=A.dtype)
    j = np.arange(L)
    for k in range(F):
        n = 2 * j - k + 1
        valid = (n >= 0) & (n < n_out)
        out[:, n[valid]] += filt[k] * A[:, j[valid]]
    return out


def _synth_chain(I, steps):
    A = I
    for filt, trim_to in steps:
        A = _idwt_half_apply(A, filt)
        if trim_to is not None and A.shape[1] == trim_to + 1:
            A = A[:, :trim_to]
    return A


def _build_stages():
    """Returns (stages, weights) where weights is [nW,128,128] f16 and
    stages is a list of dicts: name, src, dst, n_out_blocks, and
    blocks[i] = list of (in_block_j, widx)."""
    f32 = np.float32
    ops = {}
    I_n = np.eye(N, dtype=f32)
    ops['A1lo'] = ('x', 'cA1', _dwt_apply(I_n, REC_LO))
    ops['A1hi'] = ('x', 'cD1', _dwt_apply(I_n, REC_HI))
    del I_n
    I1 = np.eye(L1, dtype=f32)
    ops['A2lo'] = ('cA1', 'cA2', _dwt_apply(I1, REC_LO))
    ops['A2hi'] = ('cA1', 'cD2', _dwt_apply(I1, REC_HI))
    I2 = np.eye(L2, dtype=f32)
    ops['A3lo'] = ('cA2', 'cA3', _dwt_apply(I2, REC_LO))
    ops['A3hi'] = ('cA2', 'cD3', _dwt_apply(I2, REC_HI))
    ops['T3'] = ('cD1', 'y3', _synth_chain(I1, [(DEC_HI, None)])[:, :N])
    ops['T2'] = ('cD2', 'y2', _synth_chain(np.eye(L2, dtype=f32),
                                           [(DEC_HI, L1), (DEC_LO, None)])[:, :N])
    I3 = np.eye(L3, dtype=f32)
    ops['T1'] = ('cD3', 'y1', _synth_chain(I3, [(DEC_HI, L2), (DEC_LO, L1),
                                                (DEC_LO, None)])[:, :N])
    ops['T0'] = ('cA3', 'y0', _synth_chain(I3, [(DEC_LO, L2), (DEC_LO, L1),
                                                (DEC_LO, None)])[:, :N])
    del I1, I2, I3

    wmap = {}
    wlist = []
    stages = []
    for name, (src, dst, T_T) in ops.items():
        inlen, outlen = T_T.shape
        ib, ob = -(-inlen // P), -(-outlen // P)
        Tp = np.zeros((ib * P, ob * P), dtype=f32)
        Tp[:inlen, :outlen] = T_T
        nzb = np.abs(Tp.reshape(ib, P, ob, P)).max(axis=(1, 3)) > 0
        blocks = []
        for i in range(ob):
            lst = []
            for j in range(ib):
                if not nzb[j, i]:
                    continue
                W = Tp[j * P:(j + 1) * P, i * P:(i + 1) * P].astype(np.float16)
                key = W.tobytes()
                if key not in wmap:
                    wmap[key] = len(wlist)
                    wlist.append(W)
                lst.append((j, wmap[key]))
            blocks.append(lst)
        stages.append(dict(name=name, src=src, dst=dst, blocks=blocks))
    weights = np.stack(wlist)  # [nW, 128, 128]
    return stages, weights


_STAGES = None
_WEIGHTS = None


def _get_stages():
    global _STAGES, _WEIGHTS
    if _STAGES is None:
        _STAGES, _WEIGHTS = _build_stages()
    return _STAGES, _WEIGHTS


# coefficient tile sizes in blocks
NB = {'x': 64, 'cA1': 33, 'cD1': 33, 'cA2': 17, 'cD2': 17, 'cA3': 9, 'cD3': 9}
BAND_IDX = {'y0': 0, 'y1': 1, 'y2': 2, 'y3': 3}
GRP = 8            # output blocks per DMA group


def build_kernel():
    stages, weights = _get_stages()
    nW = weights.shape[0]
    # weights as [128, nW*128] (partition-major: W[k, m] at part k, col widx*128+m)
    wflat = np.ascontiguousarray(weights.transpose(1, 0, 2).reshape(P, nW * P)).astype(np.float16)

    nc = bass.Bass(trn_type="TRN2")
    x_d = nc.dram_tensor("x", [N, S], F16, kind="ExternalInput").ap()
    y_d = nc.dram_tensor("y", [4, N, S], F16, kind="ExternalOutput").ap()
    w_d = nc.inline_tensor(wflat, name="wts").ap()

    sinks = []
    eng_last = {}
    eng_cost = {'vector': 0.0, 'scalar': 0.0}
    eng_rate = {'vector': 1.05, 'scalar': 0.85}
    eng_fix = {'vector': 170.0, 'scalar': 220.0}

    with tile.TileContext(nc) as tc:
        with tc.tile_pool(name="ded", bufs=1) as ded, \
             tc.tile_pool(name="stg", bufs=8) as stg, \
             tc.tile_pool(name="psum", bufs=7, space="PSUM") as psum:

            wsb = ded.tile([P, nW * P], F16, tag="wsb")
            sinks.append(nc.sync.dma_start(wsb, w_d))

            tiles = {}
            for nm, nb in NB.items():
                tiles[nm] = ded.tile([P, nb * S], F16, tag=nm, name=nm)
            # load x: 8 chunks of 8 blocks each
            for g in range(8):
                src = x_d[g * 1024:(g + 1) * 1024, :].rearrange(
                    "(b p) s -> p b s", p=P)
                dst = tiles['x'][:, g * 8 * S:(g + 1) * 8 * S].rearrange(
                    "p (b s) -> p b s", s=S)
                sinks.append(nc.sync.dma_start(dst, src))

            def pick_engine():
                return min(eng_cost, key=eng_cost.get)

            def do_copy(dst_ap, src_ap, width):
                e = pick_engine()
                eng_cost[e] += width * eng_rate[e] + eng_fix[e]
                if e == 'vector':
                    eng_last[e] = nc.vector.tensor_copy(dst_ap, src_ap)
                elif e == 'gpsimd':
                    eng_last[e] = nc.gpsimd.tensor_copy(dst_ap, src_ap)
                else:
                    eng_last[e] = nc.scalar.mul(dst_ap, src_ap, 1.0)

            order = ['A1lo', 'A1hi', 'T3', 'A2lo', 'A2hi', 'T2',
                     'A3lo', 'A3hi', 'T1', 'T0']
            smap = {s3['name']: s3 for s3 in stages}
            for st in [smap[o] for o in order]:
                src_t = tiles[st['src']]
                is_band = st['dst'] in BAND_IDX
                blocks = st['blocks']
                ob = len(blocks)
                if is_band:
                    band = BAND_IDX[st['dst']]
                    for g in range(ob // GRP):
                        stgt = stg.tile([P, GRP * S], F16, tag="stg", name="stgt")
                        for t in range(GRP // 2):
                            ps = psum.tile([P, 2 * S], F32, tag="ps", name="ps")
                            for h in range(2):
                                i = g * GRP + 2 * t + h
                                lst = blocks[i]
                                for q, (j, widx) in enumerate(lst):
                                    nc.tensor.matmul(
                                        ps[:, h * S:(h + 1) * S],
                                        wsb[:, widx * P:(widx + 1) * P],
                                        src_t[:, j * S:(j + 1) * S],
                                        start=(q == 0), stop=(q == len(lst) - 1))
                            do_copy(stgt[:, 2 * t * S:(2 * t + 2) * S], ps, 2 * S)
                        dma_dst = y_d[band, g * GRP * P:(g + 1) * GRP * P, :] \
                            .rearrange("(b p) s -> p b s", p=P)
                        dma_src = stgt.rearrange("p (b s) -> p b s", s=S)
                        sinks.append(nc.sync.dma_start(dma_dst, dma_src))
                else:
                    dst_t = tiles[st['dst']]
                    for t in range(-(-ob // 2)):
                        ps = psum.tile([P, 2 * S], F32, tag="ps", name="ps")
                        nh = min(2, ob - 2 * t)
                        for h in range(nh):
                            i = 2 * t + h
                            lst = blocks[i]
                            for q, (j, widx) in enumerate(lst):
                                nc.tensor.matmul(
                                    ps[:, h * S:(h + 1) * S],
                                    wsb[:, widx * P:(widx + 1) * P],
                                    src_t[:, j * S:(j + 1) * S],
                                    start=(q == 0), stop=(q == len(lst) - 1))
                        do_copy(dst_t[:, 2 * t * S:(2 * t + nh) * S],
                                ps[:, :nh * S], nh * S)

            tc.no_sync_barrier()
            for s2 in sinks + list(eng_last.values()):
                nn = nc.sync.nop()
                add_dep_helper(nn.ins, s2.ins, reason="tail absorb")
    # PE is hardware-decoded: a Matmult may carry at most one sync wait.
    # Move extra waits onto the paired (SW-decoded) Ldweights, then split
    # any remaining multi-wait instructions via event semaphores.
    import bass_rust
    bass_rust.move_matmul_waits_to_ldweights(nc.m)
    bass_rust.generate_event_semaphores(nc)
    return nc


_NC_CACHE = None


def run_full(x_full, trace=False):
    from concourse.bass_utils import run_bass_kernel_spmd
    global _NC_CACHE
    B, C, n = x_full.shape
    xf = np.ascontiguousarray(x_full.reshape(B * C, n).astype(np.float32))
    n_cores = 8
    if _NC_CACHE is None:
        _NC_CACHE = build_kernel()
    nc = _NC_CACHE
    in_maps = []
    for i in range(n_cores):
        shard = xf[i * S:(i + 1) * S]                    # (256, 8192)
        xt = np.ascontiguousarray(shard.T.astype(np.float16))  # (8192, 256)
        in_maps.append({"x": xt})
    res = run_bass_kernel_spmd(nc, in_maps, core_ids=list(range(n_cores)),
                               trace=trace)
    bands = np.empty((4, B * C, n), dtype=np.float32)
    for i in range(n_cores):
        y = np.asarray(res.results[i]["y"])              # (4, 8192, 256) f16
        bands[:, i * S:(i + 1) * S, :] = y.transpose(0, 2, 1).astype(np.float32)
    out = tuple(bands[j].reshape(B, C, n) for j in range(4))
    return out, res


def kernel(x):
    out, _ = run_full(np.asarray(x))
    return out
